# revision 9
# baseline (speedup 1.0000x reference)
"""DigitCaps (B=32, O=1, I=4096, V=512, D=8) Trainium2 kernel.

Math: with O==1, softmax over the out-capsule axis is identically 1.0,
so all routing iterations collapse.  The whole module reduces to

    s[b,v]   = sum_{i,d} W[0,i,v,d] * x[b,i,d]        (the only heavy op)
    sq[b]    = sum_v s[b,v]^2
    out[b,v] = s * sq / ((1+sq)*sqrt(sq))             (squash)
    return (out[:,None,:], out[:,None,:])             (t == outputs)

Device strategy: shard i (4096 in-capsules) across 8 cores, 512 each.
Per core this is a [K=4096] x [B=32, V=512] contraction:
    s_c[b,v] = sum_k xT[k,b] * WT[k,v],   k = (i_local, d)
done as 32 accumulating matmuls (lhsT = xT k-tile [128,32] stationary,
rhs = WT k-tile [128,512] moving) into one PSUM bank [32,512].
W is laid out host-side into contiguous 1MB chunks so every DMA is a
big linear read; the 8 chunk loads overlap the matmul stream.
The 8 partial s_c are summed on host (64KB each) and squashed there.
"""

import numpy as np

B = 32
I = 4096
V = 512
D = 8
NCORES = 8
I_LOC = I // NCORES            # 512 in-caps per core
K_LOC = I_LOC * D              # 4096 contraction elements per core
KT = K_LOC // 128              # 32 k-tiles of 128
# W is shipped per-core as contiguous chunks of k-tiles.  Small leading
# chunks shorten the DMA->matmul pipeline prologue; total input DMA count
# (1 for x + len(CHUNK_KTS)) must stay <= 7 so the output store gets a
# fresh HWDGE lane (walrus allows only one sync wait on a DMACopy).
CHUNK_KTS = [2, 2, 4, 8, 8, 8]
assert sum(CHUNK_KTS) == KT

# matmul operand dtype: "float32" (exact, 4 cyc/row) or "float32r"
# (same 4-byte data, 1 cyc/row at N>=256) or "bfloat16" (half traffic)
MM_DTYPE = "float32"

_RUNNER = None


def _build_nc(mm_dtype: str):
    import concourse.bacc as bacc
    import concourse.mybir as mybir
    import concourse.tile as tile

    dt = getattr(mybir.dt, mm_dtype)
    nc = bacc.Bacc(trn_type="TRN2")
    x_d = nc.dram_tensor("xt_in", [128, KT, B], dt, kind="ExternalInput")
    w_d = nc.dram_tensor("wt_in", [K_LOC * V], dt, kind="ExternalInput")
    o_d = nc.dram_tensor("part_out", [B, V], mybir.dt.float32, kind="ExternalOutput")

    with tile.TileContext(nc) as tc:
        with (
            tc.tile_pool(name="xp", bufs=1) as xp,
            tc.tile_pool(name="wp", bufs=1) as wp,
            tc.tile_pool(name="pp", bufs=1, space="PSUM") as pp,
            tc.tile_pool(name="op", bufs=1) as op,
        ):
            xt = xp.tile([128, KT, B], dt)
            nc.sync.dma_start(xt[:], x_d[:])
            wts = []
            off = 0
            for g, kts in enumerate(CHUNK_KTS):
                nwords = 128 * kts * V
                wt = wp.tile([128, kts, V], dt, tag=f"w{g}")
                nc.sync.dma_start(
                    wt[:],
                    w_d[off : off + nwords].rearrange(
                        "(p j v) -> p j v", p=128, j=kts
                    ),
                )
                wts.append(wt)
                off += nwords
            # Absorb the xt-DMA dependency into a throwaway matmul so the
            # first real matmul carries only one sync wait (walrus limit on
            # the fp32 self-loading Matmult's LDW slot).
            scratch = pp.tile([B, 1], mybir.dt.float32, tag="scratch")
            nc.tensor.matmul(scratch[:], xt[:, 0, :], xt[:, 0, 0:1],
                             start=True, stop=True)
            ps = pp.tile([B, V], mybir.dt.float32)
            kt = 0
            for g, kts in enumerate(CHUNK_KTS):
                for j in range(kts):
                    nc.tensor.matmul(
                        ps[:],
                        xt[:, kt, :],
                        wts[g][:, j, :],
                        start=(kt == 0),
                        stop=(kt == KT - 1),
                    )
                    kt += 1
            ot = op.tile([B, V], mybir.dt.float32)
            nc.vector.tensor_copy(ot[:], ps[:])
            nc.sync.dma_start(o_d[:], ot[:])

    nc.finalize()
    return nc


class _Runner:
    """Cached jit(shard_map) executor for the SPMD bass kernel.

    Mirrors concourse.bass2jax.run_bass_via_pjrt's multi-core path, but
    keeps the jitted callable so repeat calls don't re-trace/re-compile.
    """

    def __init__(self, nc, n_cores=NCORES):
        import jax
        import concourse.mybir as mybir
        from concourse import bass2jax
        from jax.experimental.shard_map import shard_map
        from jax.sharding import Mesh, PartitionSpec

        bass2jax.install_neuronx_cc_hook()
        self.nc = nc
        self.n_cores = n_cores
        partition_name = nc.partition_id_tensor.name if nc.partition_id_tensor else None

        in_names, out_names, out_avals, zero_shapes = [], [], [], []
        for alloc in nc.m.functions[0].allocations:
            if not isinstance(alloc, mybir.MemoryLocationSet):
                continue
            name = alloc.memorylocations[0].name
            if alloc.kind == "ExternalInput":
                if name != partition_name:
                    in_names.append(name)
            elif alloc.kind == "ExternalOutput":
                shape = tuple(alloc.tensor_shape)
                np_dt = mybir.dt.np(alloc.dtype)
                out_avals.append(jax.core.ShapedArray(shape, np_dt))
                out_names.append(name)
                zero_shapes.append((shape, np_dt))

        n_params = len(in_names)
        n_outs = len(out_avals)
        all_in_names = list(in_names) + list(out_names)
        if partition_name is not None:
            all_in_names.append(partition_name)

        def _body(*args):
            operands = list(args)
            if partition_name is not None:
                operands.append(bass2jax.partition_id_tensor())
            outs = bass2jax._bass_exec_p.bind(
                *operands,
                out_avals=tuple(out_avals),
                in_names=tuple(all_in_names),
                out_names=tuple(out_names),
                lowering_input_output_aliases=(),
                sim_require_finite=True,
                sim_require_nnan=True,
                nc=nc,
            )
            return tuple(outs)

        devices = jax.devices()[:n_cores]
        assert len(devices) == n_cores
        self.mesh = Mesh(np.asarray(devices), ("core",))
        in_specs = (PartitionSpec("core"),) * (n_params + n_outs)
        out_specs = (PartitionSpec("core"),) * n_outs
        donate = tuple(range(n_params, n_params + n_outs))
        self._jit = jax.jit(
            shard_map(
                _body,
                mesh=self.mesh,
                in_specs=in_specs,
                out_specs=out_specs,
                check_rep=False,
            ),
            donate_argnums=donate,
            keep_unused=True,
        )
        self.in_names = in_names
        self.out_names = out_names
        self.out_avals = out_avals
        self.zero_shapes = zero_shapes

    def concat_inputs(self, in_maps):
        return [
            np.concatenate([np.asarray(m[name]) for m in in_maps], axis=0)
            for name in self.in_names
        ]

    def zeros(self):
        return [
            np.zeros((self.n_cores * s[0], *s[1:]), d) for (s, d) in self.zero_shapes
        ]

    def execute(self, concat_in):
        """Run once; returns list of global (concat) np output arrays."""
        out_arrs = self._jit(*concat_in, *self.zeros())
        return [np.asarray(a) for a in out_arrs]

    def __call__(self, in_maps):
        outs = self.execute(self.concat_inputs(in_maps))
        res = []
        for c in range(self.n_cores):
            res.append(
                {
                    name: outs[i].reshape(self.n_cores, *self.out_avals[i].shape)[c]
                    for i, name in enumerate(self.out_names)
                }
            )
        return res


def _get_runner():
    global _RUNNER
    if _RUNNER is None:
        _RUNNER = _Runner(_build_nc(MM_DTYPE))
    return _RUNNER


def _np_dtype_for(mm_dtype: str):
    if mm_dtype == "bfloat16":
        import ml_dtypes

        return np.dtype(ml_dtypes.bfloat16)
    return np.float32


def prepare_in_maps(x: np.ndarray, W: np.ndarray):
    """Host-side shard + relayout. Returns in_maps (one dict per core)."""
    np_dt = _np_dtype_for(MM_DTYPE)
    x = np.ascontiguousarray(np.asarray(x, dtype=np.float32))
    W = np.ascontiguousarray(np.asarray(W, dtype=np.float32))
    # WT[k, v] with k = i*D + d :  [I*D, V]
    WT = W.reshape(I, V, D).transpose(0, 2, 1).reshape(I * D, V)
    # xT[k, b] : [I*D, B]
    xT = x.transpose(1, 2, 0).reshape(I * D, B)
    in_maps = []
    for c in range(NCORES):
        wc = WT[c * K_LOC : (c + 1) * K_LOC]  # [4096, 512], k-major
        wc_t = wc.reshape(KT, 128, V)  # [kt, p, v]
        blocks = []
        kt0 = 0
        for kts in CHUNK_KTS:
            # chunk block layout: [p, j, v] contiguous
            blk = wc_t[kt0 : kt0 + kts].transpose(1, 0, 2)
            blocks.append(np.ascontiguousarray(blk, dtype=np_dt).reshape(-1))
            kt0 += kts
        wc_flat = np.concatenate(blocks)
        xc = xT[c * K_LOC : (c + 1) * K_LOC]  # [4096, 32]
        xc = np.ascontiguousarray(
            xc.reshape(KT, 128, B).transpose(1, 0, 2), dtype=np_dt
        )
        in_maps.append({"xt_in": xc, "wt_in": wc_flat})
    return in_maps


def finalize(partials):
    """Sum per-core partials, apply squash, build (t, outputs)."""
    s = np.zeros((B, V), dtype=np.float64)
    for p in partials:
        s += p.astype(np.float64)
    sq = (s * s).sum(axis=1, keepdims=True)  # [B,1]
    out = s * sq / ((1.0 + sq) * np.sqrt(sq))  # [B,V]
    out = out.astype(np.float32).reshape(B, 1, V)
    t = out.copy()
    return (t, out)


def kernel(x: np.ndarray, W: np.ndarray):
    runner = _get_runner()
    in_maps = prepare_in_maps(x, W)
    results = runner(in_maps)
    partials = [results[c]["part_out"] for c in range(NCORES)]
    return finalize(partials)


# revision 13
# speedup vs baseline: 634.0054x; 634.0054x over previous
"""DigitCaps (B=32, O=1, I=4096, V=512, D=8) Trainium2 kernel.

Math: with O==1, softmax over the out-capsule axis is identically 1.0,
so all routing iterations collapse.  The whole module reduces to

    s[b,v]   = sum_{i,d} W[0,i,v,d] * x[b,i,d]        (the only heavy op)
    sq[b]    = sum_v s[b,v]^2
    out[b,v] = s * sq / ((1+sq)*sqrt(sq))             (squash)
    return (out[:,None,:], out[:,None,:])             (t == outputs)

Device strategy: shard i (4096 in-capsules) across 8 cores, 512 each.
Per core this is a [K=4096] x [B=32, V=512] contraction:
    s_c[b,v] = sum_k xT[k,b] * WT[k,v],   k = (i_local, d)
done as 32 accumulating matmuls (lhsT = xT k-tile [128,32] stationary,
rhs = WT k-tile [128,512] moving) into one PSUM bank [32,512].
W is laid out host-side into contiguous 1MB chunks so every DMA is a
big linear read; the 8 chunk loads overlap the matmul stream.
The 8 partial s_c are summed on host (64KB each) and squashed there.
"""

import numpy as np

B = 32
I = 4096
V = 512
D = 8
NCORES = 8
I_LOC = I // NCORES            # 512 in-caps per core
K_LOC = I_LOC * D              # 4096 contraction elements per core
KT = K_LOC // 128              # 32 k-tiles of 128
# W is shipped per-core as contiguous chunks of k-tiles.  Small leading
# chunks shorten the DMA->matmul pipeline prologue; total input DMA count
# (1 for x + len(CHUNK_KTS)) must stay <= 7 so the output store gets a
# fresh HWDGE lane (walrus allows only one sync wait on a DMACopy).
CHUNK_KTS = [4, 4, 8, 8, 8]
assert sum(CHUNK_KTS) == KT

# matmul operand dtype.  float16 halves DMA traffic (the bottleneck) and
# runs the PE at 1 cyc/row; rel err ~5e-4 vs the 2e-2 gate (fp16 keeps 10
# mantissa bits and |W|~0.05, |x|~5 are far inside fp16 range).
# "float32" (exact, 4 cyc/row) kept as a fallback switch.
MM_DTYPE = "float16"

_RUNNER = None


def _emit_body(nc, mybir, dt, x_d, w_d, o_d, xp, wp, pp, op):
    xt = xp.tile([128, KT, B], dt)
    nc.sync.dma_start(xt[:], x_d[:])
    wts = []
    off = 0
    for g, kts in enumerate(CHUNK_KTS):
        nwords = 128 * kts * V
        wt = wp.tile([128, kts, V], dt, tag=f"w{g}")
        nc.sync.dma_start(
            wt[:],
            w_d[off : off + nwords].rearrange("(p j v) -> p j v", p=128, j=kts),
        )
        wts.append(wt)
        off += nwords
    # Absorb the xt-DMA dependency into a throwaway matmul so the
    # first real matmul carries only one sync wait (walrus limit on
    # the fp32 self-loading Matmult's LDW slot).
    scratch = pp.tile([B, 1], mybir.dt.float32, tag="scratch")
    nc.tensor.matmul(scratch[:], xt[:, 0, :], xt[:, 0, 0:1], start=True, stop=True)
    ps = pp.tile([B, V], mybir.dt.float32)
    kt = 0
    for g, kts in enumerate(CHUNK_KTS):
        for j in range(kts):
            nc.tensor.matmul(
                ps[:],
                xt[:, kt, :],
                wts[g][:, j, :],
                start=(kt == 0),
                stop=(kt == KT - 1),
            )
            kt += 1
    ot = op.tile([B, V], mybir.dt.float32)
    nc.vector.tensor_copy(ot[:], ps[:])
    nc.sync.dma_start(o_d[:], ot[:])


def _build_nc(mm_dtype: str, reps: int = 1):
    import concourse.bacc as bacc
    import concourse.mybir as mybir
    import concourse.tile as tile

    dt = getattr(mybir.dt, mm_dtype)
    nc = bacc.Bacc(trn_type="TRN2")
    x_d = nc.dram_tensor("xt_in", [128, KT, B], dt, kind="ExternalInput")
    w_d = nc.dram_tensor("wt_in", [K_LOC * V], dt, kind="ExternalInput")
    o_d = nc.dram_tensor("part_out", [B, V], mybir.dt.float32, kind="ExternalOutput")

    with tile.TileContext(nc) as tc:
        with (
            tc.tile_pool(name="xp", bufs=1) as xp,
            tc.tile_pool(name="wp", bufs=1) as wp,
            tc.tile_pool(name="pp", bufs=1, space="PSUM") as pp,
            tc.tile_pool(name="op", bufs=1) as op,
        ):
            if reps == 1:
                _emit_body(nc, mybir, dt, x_d, w_d, o_d, xp, wp, pp, op)
            else:
                with tc.For_i(0, reps, 1):
                    _emit_body(nc, mybir, dt, x_d, w_d, o_d, xp, wp, pp, op)

    nc.finalize()
    return nc


class _Runner:
    """Cached jit(shard_map) executor for the SPMD bass kernel.

    Mirrors concourse.bass2jax.run_bass_via_pjrt's multi-core path, but
    keeps the jitted callable so repeat calls don't re-trace/re-compile.
    """

    def __init__(self, nc, n_cores=NCORES):
        import jax
        import concourse.mybir as mybir
        from concourse import bass2jax
        from jax.experimental.shard_map import shard_map
        from jax.sharding import Mesh, PartitionSpec

        bass2jax.install_neuronx_cc_hook()
        self.nc = nc
        self.n_cores = n_cores
        partition_name = nc.partition_id_tensor.name if nc.partition_id_tensor else None

        in_names, out_names, out_avals, zero_shapes = [], [], [], []
        for alloc in nc.m.functions[0].allocations:
            if not isinstance(alloc, mybir.MemoryLocationSet):
                continue
            name = alloc.memorylocations[0].name
            if alloc.kind == "ExternalInput":
                if name != partition_name:
                    in_names.append(name)
            elif alloc.kind == "ExternalOutput":
                shape = tuple(alloc.tensor_shape)
                np_dt = mybir.dt.np(alloc.dtype)
                out_avals.append(jax.core.ShapedArray(shape, np_dt))
                out_names.append(name)
                zero_shapes.append((shape, np_dt))

        n_params = len(in_names)
        n_outs = len(out_avals)
        all_in_names = list(in_names) + list(out_names)
        if partition_name is not None:
            all_in_names.append(partition_name)

        def _body(*args):
            operands = list(args)
            if partition_name is not None:
                operands.append(bass2jax.partition_id_tensor())
            outs = bass2jax._bass_exec_p.bind(
                *operands,
                out_avals=tuple(out_avals),
                in_names=tuple(all_in_names),
                out_names=tuple(out_names),
                lowering_input_output_aliases=(),
                sim_require_finite=True,
                sim_require_nnan=True,
                nc=nc,
            )
            return tuple(outs)

        devices = jax.devices()[:n_cores]
        assert len(devices) == n_cores
        self.mesh = Mesh(np.asarray(devices), ("core",))
        in_specs = (PartitionSpec("core"),) * (n_params + n_outs)
        out_specs = (PartitionSpec("core"),) * n_outs
        donate = tuple(range(n_params, n_params + n_outs))
        self._jit = jax.jit(
            shard_map(
                _body,
                mesh=self.mesh,
                in_specs=in_specs,
                out_specs=out_specs,
                check_rep=False,
            ),
            donate_argnums=donate,
            keep_unused=True,
        )
        self.in_names = in_names
        self.out_names = out_names
        self.out_avals = out_avals
        self.zero_shapes = zero_shapes

    def concat_inputs(self, in_maps):
        return [
            np.concatenate([np.asarray(m[name]) for m in in_maps], axis=0)
            for name in self.in_names
        ]

    def zeros(self):
        return [
            np.zeros((self.n_cores * s[0], *s[1:]), d) for (s, d) in self.zero_shapes
        ]

    def execute(self, concat_in):
        """Run once; returns list of global (concat) np output arrays."""
        out_arrs = self._jit(*concat_in, *self.zeros())
        return [np.asarray(a) for a in out_arrs]

    def __call__(self, in_maps):
        outs = self.execute(self.concat_inputs(in_maps))
        res = []
        for c in range(self.n_cores):
            res.append(
                {
                    name: outs[i].reshape(self.n_cores, *self.out_avals[i].shape)[c]
                    for i, name in enumerate(self.out_names)
                }
            )
        return res


def _get_runner():
    global _RUNNER
    if _RUNNER is None:
        _RUNNER = _Runner(_build_nc(MM_DTYPE))
    return _RUNNER


def _np_dtype_for(mm_dtype: str):
    if mm_dtype == "bfloat16":
        import ml_dtypes

        return np.dtype(ml_dtypes.bfloat16)
    if mm_dtype == "float16":
        return np.dtype(np.float16)
    return np.float32


def prepare_in_maps(x: np.ndarray, W: np.ndarray):
    """Host-side shard + relayout. Returns in_maps (one dict per core)."""
    np_dt = _np_dtype_for(MM_DTYPE)
    x = np.ascontiguousarray(np.asarray(x, dtype=np.float32))
    W = np.ascontiguousarray(np.asarray(W, dtype=np.float32))
    # WT[k, v] with k = i*D + d :  [I*D, V]
    WT = W.reshape(I, V, D).transpose(0, 2, 1).reshape(I * D, V)
    # xT[k, b] : [I*D, B]
    xT = x.transpose(1, 2, 0).reshape(I * D, B)
    in_maps = []
    for c in range(NCORES):
        wc = WT[c * K_LOC : (c + 1) * K_LOC]  # [4096, 512], k-major
        wc_t = wc.reshape(KT, 128, V)  # [kt, p, v]
        blocks = []
        kt0 = 0
        for kts in CHUNK_KTS:
            # chunk block layout: [p, j, v] contiguous
            blk = wc_t[kt0 : kt0 + kts].transpose(1, 0, 2)
            blocks.append(np.ascontiguousarray(blk, dtype=np_dt).reshape(-1))
            kt0 += kts
        wc_flat = np.concatenate(blocks)
        xc = xT[c * K_LOC : (c + 1) * K_LOC]  # [4096, 32]
        xc = np.ascontiguousarray(
            xc.reshape(KT, 128, B).transpose(1, 0, 2), dtype=np_dt
        )
        in_maps.append({"xt_in": xc, "wt_in": wc_flat})
    return in_maps


def finalize(partials):
    """Sum per-core partials, apply squash, build (t, outputs)."""
    s = np.zeros((B, V), dtype=np.float64)
    for p in partials:
        s += p.astype(np.float64)
    sq = (s * s).sum(axis=1, keepdims=True)  # [B,1]
    out = s * sq / ((1.0 + sq) * np.sqrt(sq))  # [B,V]
    out = out.astype(np.float32).reshape(B, 1, V)
    t = out.copy()
    return (t, out)


def kernel(x: np.ndarray, W: np.ndarray):
    runner = _get_runner()
    in_maps = prepare_in_maps(x, W)
    results = runner(in_maps)
    partials = [results[c]["part_out"] for c in range(NCORES)]
    return finalize(partials)
